# revision 36
# baseline (speedup 1.0000x reference)
"""Multi-head attention (B=2, S=2048, D=1024, H=16, d_k=64) on 8 Trainium2
NeuronCores.

Sharding: core = b * 4 + g  (b = batch, g = head-group of 4 heads).
Each core projects Q/K/V for its 4 heads (column-sharded Wq/Wk/Wv), runs
masked softmax attention, and computes a partial output projection with the
row-shard of Wo.  The host sums the 4 partials per batch and adds bo.

Mask handling: the key mask is applied on the host by gathering only the
unmasked key columns (exp(-1e9) == 0 exactly in fp32, so the reference's
masked softmax is exactly a softmax over the unmasked keys).  Keys are
padded to a multiple of 128; pad keys get zero "ones" columns and zero V
rows so they contribute nothing.

Layout / schedule (v3, ~1.9x over the f32 baseline):
  * all DRAM traffic in bf16 (x, weights, y partials) — halves DMA time and
    enables FWL fast weight loads on every matmul; x tensors are fully
    resident in SBUF, DMA'd in dependency order (xk quarters first, then
    xv, then xq) so each projection stage's data lands just before it runs.
  * projections run K, V, Q; K is kept merged per j-chunk: kt2[jc]
    [128, k_pad] holds head 2jc on partitions 0:64 and head 2jc+1 on
    64:128.  Score matmuls contract over 64 partitions and run as
    row-tiled pairs (tile_position rows 0 / 64) concurrently in the PE.
  * V stationary tiles carry a 64-wide ones block per head, so the PV
    matmul replicates the softmax denominator across 64 PSUM partitions:
      even head: [V(64) | ones(64)]  -> aug rows 0:64 data, 64:128 denom
      odd head:  [ones(64) | V(64)]  -> aug rows 0:64 denom, 64:128 data
    Normalize is pure DVE: a cross-half plain copy (legal on HW), the
    custom reciprocal at base partition 0 (REQUIRED: it breaks or crashes
    at other bases), another cross-half copy, and a multiply.  ScalarE
    does nothing but Exp, back to back: the attention phase is scalar-
    bound at ~98% ACT occupancy (the floor for this algorithm).
  * attention runs per 512-wide q granule x head-pair block; ps tiles
    [128,1024] hold both heads of a pair so one 1024-wide exp instruction
    covers them.  PSUM: ps 2x2 banks double-buffered + aug 4x1 banks.
  * the output projection runs as a PE-only tail after attention (any
    PE insert into the scalar-bound attention gaps the exp stream 1:1),
    drawing its psum from the then-idle aug banks, with y written back
    in 2-tile batches that dribble out during the tail.
"""

import sys
import types

sys.path.insert(0, "/opt/trn_rl_repo")

# The axon boot script installs an NTFF-profiling hook only if
# ``antenv.axon_hooks`` is importable; this image's antenv lacks it, so
# provide it before anything triggers jax/axon platform registration.
if "antenv.axon_hooks" not in sys.modules:
    _hooks_mod = types.ModuleType("antenv.axon_hooks")
    _hooks_mod._hook = None

    def _set_hook(h, _m=_hooks_mod):
        _m._hook = h

    def _get_hook(_m=_hooks_mod):
        return _m._hook

    _hooks_mod.set_axon_ntff_profile_hook = _set_hook
    _hooks_mod.get_axon_ntff_profile_hook = _get_hook
    sys.modules["antenv.axon_hooks"] = _hooks_mod
    try:
        import antenv as _antenv

        _antenv.axon_hooks = _hooks_mod
    except ImportError:
        pass

import ml_dtypes
import numpy as np

import concourse.bass as bass  # noqa: F401  (import keeps bass registered)
import concourse.mybir as mybir
import concourse.tile as tile
from concourse import bacc

F32 = mybir.dt.float32
BF16 = mybir.dt.bfloat16
AF = mybir.ActivationFunctionType
ALU = mybir.AluOpType
BF16NP = ml_dtypes.bfloat16

D = 1024  # model dim
S = 2048  # sequence length
HL = 4  # heads per core
DK = 64  # head dim
J = HL * DK  # 256 projected dims per core
DC = D // 128  # 8 contraction chunks
JC = J // 128  # 2 j-chunks
B = 2
GROUPS = 4
NCORES = B * GROUPS
QG = 512  # q granule width
NQG = S // QG


def emit_outproj_granule(nc, ap, ypool, at_sb, wo_sb, y_r, qg):
    """Output projection for one 512-row q granule, run as a PE tail.
    psy halves come from the aug tag (idle once attention ends, depth-4
    rotation) so the tail never serializes on its own drain copies."""
    for half in range(2):
        yt = ypool.tile([128, 2, 1024], BF16, tag="yt", name="yt")
        for i in range(2):
            qt = qg * 4 + half * 2 + i
            for mc in range(2):
                psy = ap.tile([128, 512], F32, tag="aug", name="psy")
                for jc in range(JC):
                    nc.tensor.matmul(
                        psy[:],
                        at_sb[jc][:, qt * 128 : (qt + 1) * 128],
                        wo_sb[:, jc, mc * 512 : (mc + 1) * 512],
                        start=(jc == 0),
                        stop=(jc == JC - 1),
                    )
                nc.vector.tensor_copy(
                    yt[:, i, mc * 512 : (mc + 1) * 512], psy[:]
                )
        nc.sync.dma_start(
            y_r[:, qg * 4 + half * 2 : qg * 4 + half * 2 + 2, :], yt[:]
        )


def build_program(kt_tiles: int):
    k_pad = kt_tiles * 128
    nc = bacc.Bacc()

    xq = nc.declare_dram_parameter("xq", [D, S], BF16, isOutput=False)
    xk = nc.declare_dram_parameter("xk", [D, k_pad], BF16, isOutput=False)
    xv = nc.declare_dram_parameter("xv", [D, k_pad], BF16, isOutput=False)
    wq = nc.declare_dram_parameter("wq", [D, J], BF16, isOutput=False)
    wk = nc.declare_dram_parameter("wk", [D, J], BF16, isOutput=False)
    wv = nc.declare_dram_parameter("wv", [D, J], BF16, isOutput=False)
    wo = nc.declare_dram_parameter("wo", [J, D], BF16, isOutput=False)
    bq = nc.declare_dram_parameter("bq", [J], F32, isOutput=False)
    bk = nc.declare_dram_parameter("bk", [J], F32, isOutput=False)
    bv = nc.declare_dram_parameter("bv", [J], F32, isOutput=False)
    kones = nc.declare_dram_parameter("kones", [k_pad], F32, isOutput=False)
    y = nc.declare_dram_parameter("y", [S, D], BF16, isOutput=True)

    with tile.TileContext(nc) as tc:
        with (
            tc.tile_pool(name="const", bufs=1) as cpool,
            tc.tile_pool(name="big", bufs=1) as big,
            tc.tile_pool(name="xin", bufs=3) as xin,
            tc.tile_pool(name="ptile", bufs=3) as ppool,
            tc.tile_pool(name="ypool", bufs=3) as ypool,
            tc.tile_pool(name="small", bufs=3) as small,
        ):
            # persistent activations (all bf16)
            qt_sb = [big.tile([128, S], BF16, tag=f"qt{jc}", name=f"qt{jc}") for jc in range(JC)]
            kt2 = [big.tile([128, k_pad], BF16, tag=f"kt{jc}", name=f"kt{jc}") for jc in range(JC)]
            at_sb = [big.tile([128, S], BF16, tag=f"at{jc}", name=f"at{jc}") for jc in range(JC)]
            v_sb = [big.tile([128, 512], BF16, tag=f"v{kt}", name=f"v{kt}") for kt in range(kt_tiles)]

            with tc.tile_pool(name="proj_psum", bufs=1, space="PSUM") as pp:
                # x tensors are fully resident; DMAs are issued in dependency
                # order (K first, then V, then Q in two halves) so each
                # projection stage's data lands just before it runs
                QC4 = DC // 4
                wk_sb = cpool.tile([128, DC, J], BF16, tag="wk")
                nc.sync.dma_start(wk_sb[:], wk.rearrange("(c p) j -> p c j", p=128))
                bk_sb = cpool.tile([128, JC], F32, tag="bk")
                nc.sync.dma_start(bk_sb[:], bk.rearrange("(c p) -> p c", p=128))
                xk_sb = big.tile([128, DC, k_pad], BF16, tag="xk")
                xk_r = xk.rearrange("(c p) k -> p c k", p=128)
                for q4 in range(4):
                    nc.sync.dma_start(
                        xk_sb[:, q4 * QC4 : (q4 + 1) * QC4, :],
                        xk_r[:, q4 * QC4 : (q4 + 1) * QC4, :],
                    )
                xv_sb = big.tile([128, DC, k_pad], BF16, tag="xv")
                nc.sync.dma_start(xv_sb[:], xv.rearrange("(c p) k -> p c k", p=128))
                wv_sb = cpool.tile([128, DC, J], BF16, tag="wv")
                nc.sync.dma_start(wv_sb[:], wv.rearrange("(c p) j -> p c j", p=128))
                bv_bc = cpool.tile([128, J], F32, tag="bv")
                nc.sync.dma_start(bv_bc[:], bv.ap()[None, :].to_broadcast((128, J)))
                kones_sb = cpool.tile([128, kt_tiles], F32, tag="kones")
                nc.sync.dma_start(kones_sb[:], kones.rearrange("(t p) -> p t", p=128))
                wq_sb = cpool.tile([128, DC, J], BF16, tag="wq")
                nc.sync.dma_start(wq_sb[:], wq.rearrange("(c p) j -> p c j", p=128))
                bq_sb = cpool.tile([128, JC], F32, tag="bq")
                nc.sync.dma_start(bq_sb[:], bq.rearrange("(c p) -> p c", p=128))
                xq_sb = big.tile([128, DC, S], BF16, tag="xq")
                xq_r = xq.rearrange("(c p) s -> p c s", p=128)
                for q4 in range(4):
                    nc.sync.dma_start(
                        xq_sb[:, q4 * QC4 : (q4 + 1) * QC4, :],
                        xq_r[:, q4 * QC4 : (q4 + 1) * QC4, :],
                    )
                wo_sb = cpool.tile([128, JC, D], BF16, tag="wo")
                nc.sync.dma_start(wo_sb[:], wo.rearrange("(c p) m -> p c m", p=128))

                # warm up the ACT exp table while ScalarE is otherwise
                # idle, so the ~2.7us first-use table load doesn't land in
                # the scalar-critical attention window
                warm = small.tile([1, 32], F32, tag="warm")
                nc.vector.memset(warm[:], 0.0)
                warm2 = small.tile([1, 32], F32, tag="warm")
                nc.scalar.activation(warm2[:], warm[:], AF.Exp, scale=1.0)

                # ---- K^T projection (merged per-chunk layout)
                kchunks = []
                off = 0
                while off < k_pad:
                    w = min(512, k_pad - off)
                    kchunks.append((off, w))
                    off += w
                psk = [
                    pp.tile([128, 512], F32, tag=f"psq{i}", name=f"psk{i}")
                    for i in range(JC * len(kchunks))
                ]
                for dc in range(DC):
                    for jc in range(JC):
                        lhsT = wk_sb[:, dc, jc * 128 : (jc + 1) * 128]
                        for i, (off, w) in enumerate(kchunks):
                            nc.tensor.matmul(
                                psk[jc * len(kchunks) + i][:, :w],
                                lhsT,
                                xk_sb[:, dc, off : off + w],
                                start=(dc == 0),
                                stop=(dc == DC - 1),
                            )
                for jc in range(JC):
                    for i, (off, w) in enumerate(kchunks):
                        nc.vector.tensor_tensor(
                            kt2[jc][:, off : off + w],
                            psk[jc * len(kchunks) + i][:, :w],
                            bk_sb[:, jc : jc + 1].to_broadcast((128, w)),
                            ALU.add,
                        )

                # ---- V natural projection (+ per-head 64-wide ones blocks) --
                for kt in range(kt_tiles):
                    psv = pp.tile([128, J], F32, tag=f"psq{kt % 2}", name="psv")
                    for dc in range(DC):
                        nc.tensor.matmul(
                            psv[:],
                            xv_sb[:, dc, kt * 128 : (kt + 1) * 128],
                            wv_sb[:, dc, :],
                            start=(dc == 0),
                            stop=(dc == DC - 1),
                        )
                    vt = v_sb[kt]
                    kcol = kones_sb[:, kt : kt + 1]
                    # pad keys have xv == 0, so psv pad rows are already 0:
                    # masking only needs to zero the bias on pad rows.
                    # bvko = bv * kones (rank-1), one op for all four heads.
                    bvko = small.tile([128, J], F32, tag="bvko")
                    nc.vector.tensor_scalar(
                        bvko[:], bv_bc[:], kcol, None, ALU.mult
                    )
                    for h in range(HL):
                        pair = h // 2
                        d0 = pair * 256 + (0 if h % 2 == 0 else 192)
                        nc.vector.tensor_tensor(
                            vt[:, d0 : d0 + DK],
                            psv[:, h * DK : (h + 1) * DK],
                            bvko[:, h * DK : (h + 1) * DK],
                            ALU.add,
                        )
                    for pair in range(JC):
                        o0 = pair * 256 + 64
                        nc.vector.tensor_copy(
                            vt[:, o0 : o0 + 128], kcol.to_broadcast((128, 128))
                        )


                # ---- Q^T projection (dc-outer; drains at the end)
                QQC = S // 512
                psq = [
                    pp.tile([128, 512], F32, tag=f"psq{i}", name=f"psq{i}")
                    for i in range(JC * QQC)
                ]
                for dc in range(DC):
                    for jc in range(JC):
                        lhsT = wq_sb[:, dc, jc * 128 : (jc + 1) * 128]
                        for qc in range(QQC):
                            nc.tensor.matmul(
                                psq[jc * QQC + qc][:],
                                lhsT,
                                xq_sb[:, dc, qc * 512 : (qc + 1) * 512],
                                start=(dc == 0),
                                stop=(dc == DC - 1),
                            )
                for qc in range(QQC):  # qc-major: granule 0 drains first
                    for jc in range(JC):
                        nc.vector.tensor_tensor(
                            qt_sb[jc][:, qc * 512 : (qc + 1) * 512],
                            psq[jc * QQC + qc][:],
                            bq_sb[:, jc : jc + 1].to_broadcast((128, 512)),
                            ALU.add,
                        )

            # ---- attention + per-granule output projection ------------------
            # Software-pipelined at (granule, pair)-block granularity: the
            # first two kt iterations of each block are emitted BEFORE the
            # previous block's psy/normalize tail, so ScalarE always has
            # buffered exp work while the in-order PE drains the lagged
            # out-projection tiles.
            with (
                tc.tile_pool(name="score_psum", bufs=2, space="PSUM") as sp,
                tc.tile_pool(name="aug_psum", bufs=4, space="PSUM") as ap,
            ):
                PIPE = 2 if kt_tiles > 3 else 0

                def att_kt(qg, pair, kt, aug_e, aug_o):
                    q0 = qg * QG
                    ps = sp.tile([128, 1024], F32, tag="ps", name="ps")
                    ksl = slice(kt * 128, (kt + 1) * 128)
                    # row-tiled concurrent score pair (K=64 each)
                    nc.tensor.matmul(
                        ps[:, 0:QG],
                        kt2[pair][0:64, ksl],
                        qt_sb[pair][0:64, q0 : q0 + QG],
                        start=True,
                        stop=True,
                    )
                    nc.tensor.matmul(
                        ps[:, QG : 2 * QG],
                        kt2[pair][64:128, ksl],
                        qt_sb[pair][64:128, q0 : q0 + QG],
                        start=True,
                        stop=True,
                    )
                    pt = ppool.tile([128, 1024], BF16, tag="pt")
                    nc.scalar.activation(pt[:], ps[:], AF.Exp, scale=0.125)
                    nc.tensor.matmul(
                        aug_e[:],
                        v_sb[kt][:, pair * 256 : pair * 256 + 128],
                        pt[:, 0:QG],
                        start=(kt == 0),
                        stop=(kt == kt_tiles - 1),
                    )
                    nc.tensor.matmul(
                        aug_o[:],
                        v_sb[kt][:, pair * 256 + 128 : pair * 256 + 256],
                        pt[:, QG : 2 * QG],
                        start=(kt == 0),
                        stop=(kt == kt_tiles - 1),
                    )

                def normalize(qg, pair, aug_e, aug_o):
                    # the custom reciprocal op only works at base partition 0
                    # on HW, but PLAIN DVE copies may cross halves — so the
                    # partition shifts are cheap vector copies, no DMAs
                    q0 = qg * QG
                    dl = small.tile([128, QG], F32, tag="dl")
                    nc.vector.tensor_copy(dl[0:64, :], aug_e[64:128, :])
                    rr = small.tile([128, QG], F32, tag="rr")
                    nc.vector.reciprocal_approx_fast(rr[0:64, :], dl[0:64, :])
                    ro = small.tile([128, QG], F32, tag="ro")
                    nc.vector.reciprocal_approx_fast(ro[0:64, :], aug_o[0:64, :])
                    rb = small.tile([128, QG], F32, tag="rb")
                    nc.vector.tensor_copy(rb[64:128, :], ro[0:64, :])
                    nc.vector.tensor_tensor(
                        at_sb[pair][0:64, q0 : q0 + QG],
                        aug_e[0:64, :],
                        rr[0:64, :],
                        ALU.mult,
                    )
                    nc.vector.tensor_tensor(
                        at_sb[pair][64:128, q0 : q0 + QG],
                        aug_o[64:128, :],
                        rb[64:128, :],
                        ALU.mult,
                    )

                prev = None  # (qg, pair, aug_e, aug_o) awaiting normalize
                for qg in range(NQG):
                    for pair in range(JC):
                        aug_e = ap.tile([128, QG], F32, tag="aug", name="aug_e")
                        aug_o = ap.tile([128, QG], F32, tag="aug", name="aug_o")
                        for kt in range(PIPE):
                            att_kt(qg, pair, kt, aug_e, aug_o)
                        if prev is not None:
                            pg, pp_, pe, po = prev
                            normalize(pg, pp_, pe, po)
                        for kt in range(PIPE, kt_tiles):
                            att_kt(qg, pair, kt, aug_e, aug_o)
                        prev = (qg, pair, aug_e, aug_o)
                pg, pp_, pe, po = prev
                normalize(pg, pp_, pe, po)
                # out-projection tail: attention is scalar-bound with no PE
                # slack, so any psy insert gaps the exp stream 1:1 — run the
                # whole projection as a PE-only tail instead
                y_r = y.rearrange("(t p) d -> p t d", p=128)
                for qg in range(NQG):
                    emit_outproj_granule(nc, ap, ypool, at_sb, wo_sb, y_r, qg)

    nc.finalize()
    return nc


_CACHE: dict = {}


def _get_program(kt_tiles: int):
    if kt_tiles not in _CACHE:
        _CACHE[kt_tiles] = build_program(kt_tiles)
    return _CACHE[kt_tiles]


def _prep_inputs(q, k, v, mask, Wq, bq, Wk, bk, Wv, bv, Wo, bo):
    """Shard + transpose + compact on the host. Returns (in_maps, kt_tiles)."""
    idx = [np.nonzero(mask[b])[0] for b in range(B)]
    s_u = max(1, max(len(i) for i in idx))
    kt_tiles = (s_u + 127) // 128
    k_pad = kt_tiles * 128

    per_batch = []
    for b in range(B):
        qT = np.ascontiguousarray(q[b].T).astype(BF16NP)  # [D, S]
        kT = np.zeros((D, k_pad), BF16NP)
        vT = np.zeros((D, k_pad), BF16NP)
        n = len(idx[b])
        kT[:, :n] = k[b].T[:, idx[b]].astype(BF16NP)
        vT[:, :n] = v[b].T[:, idx[b]].astype(BF16NP)
        ko = np.zeros((k_pad,), np.float32)
        ko[:n] = 1.0
        per_batch.append((qT, kT, vT, ko))

    in_maps = []
    for core in range(NCORES):
        b, g = divmod(core, GROUPS)
        j0 = g * J
        qT, kT, vT, ko = per_batch[b]
        in_maps.append(
            {
                "xq": qT,
                "xk": kT,
                "xv": vT,
                "wq": np.ascontiguousarray(Wq[j0 : j0 + J, :].T).astype(BF16NP),
                "wk": np.ascontiguousarray(Wk[j0 : j0 + J, :].T).astype(BF16NP),
                "wv": np.ascontiguousarray(Wv[j0 : j0 + J, :].T).astype(BF16NP),
                "wo": np.ascontiguousarray(Wo[:, j0 : j0 + J].T).astype(BF16NP),
                "bq": np.ascontiguousarray(bq[j0 : j0 + J]).astype(np.float32),
                "bk": np.ascontiguousarray(bk[j0 : j0 + J]).astype(np.float32),
                "bv": np.ascontiguousarray(bv[j0 : j0 + J]).astype(np.float32),
                "kones": ko,
            }
        )
    return in_maps, kt_tiles


def run(inputs: dict, trace: bool = False):
    """Run the sharded kernel; returns (output [B,S,D] f32, BassKernelResults)."""
    from concourse.bass_utils import run_bass_kernel_spmd

    inputs = {k: np.asarray(v) for k, v in inputs.items()}
    in_maps, kt_tiles = _prep_inputs(**inputs)
    nc = _get_program(kt_tiles)
    res = run_bass_kernel_spmd(nc, in_maps, list(range(NCORES)), trace=trace)
    bo = inputs["bo"].astype(np.float32)
    out = np.empty((B, S, D), np.float32)
    for b in range(B):
        acc = np.zeros((S, D), np.float64)
        for g in range(GROUPS):
            acc += np.asarray(res.results[b * GROUPS + g]["y"], dtype=np.float64)
        out[b] = (acc + bo[None, :]).astype(np.float32)
    return out, res


def kernel(**inputs) -> np.ndarray:
    out, _ = run(inputs, trace=False)
    return out


# revision 37
# speedup vs baseline: 1.0067x; 1.0067x over previous
"""Multi-head attention (B=2, S=2048, D=1024, H=16, d_k=64) on 8 Trainium2
NeuronCores.

Sharding: core = b * 4 + g  (b = batch, g = head-group of 4 heads).
Each core projects Q/K/V for its 4 heads (column-sharded Wq/Wk/Wv), runs
masked softmax attention, and computes a partial output projection with the
row-shard of Wo.  The host sums the 4 partials per batch and adds bo.

Mask handling: the key mask is applied on the host by gathering only the
unmasked key columns (exp(-1e9) == 0 exactly in fp32, so the reference's
masked softmax is exactly a softmax over the unmasked keys).  Keys are
padded to a multiple of 128; pad keys get zero "ones" columns and zero V
rows so they contribute nothing.

Layout / schedule (v3, ~1.9x over the f32 baseline):
  * all DRAM traffic in bf16 (x, weights, y partials) — halves DMA time and
    enables FWL fast weight loads on every matmul; x tensors are fully
    resident in SBUF, DMA'd in dependency order (xk quarters first, then
    xv, then xq) so each projection stage's data lands just before it runs.
  * projections run K, V, Q; K is kept merged per j-chunk: kt2[jc]
    [128, k_pad] holds head 2jc on partitions 0:64 and head 2jc+1 on
    64:128.  Score matmuls contract over 64 partitions and run as
    row-tiled pairs (tile_position rows 0 / 64) concurrently in the PE.
  * V stationary tiles carry a 64-wide ones block per head, so the PV
    matmul replicates the softmax denominator across 64 PSUM partitions:
      even head: [V(64) | ones(64)]  -> aug rows 0:64 data, 64:128 denom
      odd head:  [ones(64) | V(64)]  -> aug rows 0:64 denom, 64:128 data
    Normalize is pure DVE: a cross-half plain copy (legal on HW), the
    custom reciprocal at base partition 0 (REQUIRED: it breaks or crashes
    at other bases), another cross-half copy, and a multiply.  ScalarE
    does nothing but Exp, back to back: the attention phase is scalar-
    bound at ~98% ACT occupancy (the floor for this algorithm).
  * attention runs per 512-wide q granule x head-pair block; ps tiles
    [128,1024] hold both heads of a pair so one 1024-wide exp instruction
    covers them.  PSUM: ps 2x2 banks double-buffered + aug 4x1 banks.
  * the output projection runs as a PE-only tail after attention (any
    PE insert into the scalar-bound attention gaps the exp stream 1:1),
    drawing its psum from the then-idle aug banks, with y written back
    in 2-tile batches that dribble out during the tail.
"""

import sys
import types

sys.path.insert(0, "/opt/trn_rl_repo")

# The axon boot script installs an NTFF-profiling hook only if
# ``antenv.axon_hooks`` is importable; this image's antenv lacks it, so
# provide it before anything triggers jax/axon platform registration.
if "antenv.axon_hooks" not in sys.modules:
    _hooks_mod = types.ModuleType("antenv.axon_hooks")
    _hooks_mod._hook = None

    def _set_hook(h, _m=_hooks_mod):
        _m._hook = h

    def _get_hook(_m=_hooks_mod):
        return _m._hook

    _hooks_mod.set_axon_ntff_profile_hook = _set_hook
    _hooks_mod.get_axon_ntff_profile_hook = _get_hook
    sys.modules["antenv.axon_hooks"] = _hooks_mod
    try:
        import antenv as _antenv

        _antenv.axon_hooks = _hooks_mod
    except ImportError:
        pass

import ml_dtypes
import numpy as np

import concourse.bass as bass  # noqa: F401  (import keeps bass registered)
import concourse.mybir as mybir
import concourse.tile as tile
from concourse import bacc

F32 = mybir.dt.float32
BF16 = mybir.dt.bfloat16
AF = mybir.ActivationFunctionType
ALU = mybir.AluOpType
BF16NP = ml_dtypes.bfloat16

D = 1024  # model dim
S = 2048  # sequence length
HL = 4  # heads per core
DK = 64  # head dim
J = HL * DK  # 256 projected dims per core
DC = D // 128  # 8 contraction chunks
JC = J // 128  # 2 j-chunks
B = 2
GROUPS = 4
NCORES = B * GROUPS
QG = 512  # q granule width
NQG = S // QG


def emit_outproj_granule(nc, ap, ypool, at_sb, wo_sb, y_r, qg):
    """Output projection for one 512-row q granule, run as a PE tail.
    psy halves come from the aug tag (idle once attention ends, depth-4
    rotation) so the tail never serializes on its own drain copies."""
    for half in range(2):
        yt = ypool.tile([128, 2, 1024], BF16, tag="yt", name="yt")
        for i in range(2):
            qt = qg * 4 + half * 2 + i
            for mc in range(2):
                psy = ap.tile([128, 512], F32, tag="aug", name="psy")
                for jc in range(JC):
                    nc.tensor.matmul(
                        psy[:],
                        at_sb[jc][:, qt * 128 : (qt + 1) * 128],
                        wo_sb[:, jc, mc * 512 : (mc + 1) * 512],
                        start=(jc == 0),
                        stop=(jc == JC - 1),
                    )
                nc.vector.tensor_copy(
                    yt[:, i, mc * 512 : (mc + 1) * 512], psy[:]
                )
        nc.sync.dma_start(
            y_r[:, qg * 4 + half * 2 : qg * 4 + half * 2 + 2, :], yt[:]
        )


def build_program(kt_tiles: int):
    k_pad = kt_tiles * 128
    nc = bacc.Bacc()

    xq = nc.declare_dram_parameter("xq", [D, S], BF16, isOutput=False)
    xk = nc.declare_dram_parameter("xk", [D, k_pad], BF16, isOutput=False)
    xv = nc.declare_dram_parameter("xv", [D, k_pad], BF16, isOutput=False)
    wq = nc.declare_dram_parameter("wq", [D, J], BF16, isOutput=False)
    wk = nc.declare_dram_parameter("wk", [D, J], BF16, isOutput=False)
    wv = nc.declare_dram_parameter("wv", [D, J], BF16, isOutput=False)
    wo = nc.declare_dram_parameter("wo", [J, D], BF16, isOutput=False)
    bq = nc.declare_dram_parameter("bq", [J], F32, isOutput=False)
    bk = nc.declare_dram_parameter("bk", [J], F32, isOutput=False)
    bv = nc.declare_dram_parameter("bv", [J], F32, isOutput=False)
    kones = nc.declare_dram_parameter("kones", [k_pad], F32, isOutput=False)
    y = nc.declare_dram_parameter("y", [S, D], BF16, isOutput=True)

    with tile.TileContext(nc) as tc:
        with (
            tc.tile_pool(name="const", bufs=1) as cpool,
            tc.tile_pool(name="big", bufs=1) as big,
            tc.tile_pool(name="xin", bufs=3) as xin,
            tc.tile_pool(name="ptile", bufs=4) as ppool,
            tc.tile_pool(name="ypool", bufs=4) as ypool,
            tc.tile_pool(name="small", bufs=3) as small,
        ):
            # persistent activations (all bf16)
            qt_sb = [big.tile([128, S], BF16, tag=f"qt{jc}", name=f"qt{jc}") for jc in range(JC)]
            kt2 = [big.tile([128, k_pad], BF16, tag=f"kt{jc}", name=f"kt{jc}") for jc in range(JC)]
            at_sb = [big.tile([128, S], BF16, tag=f"at{jc}", name=f"at{jc}") for jc in range(JC)]
            v_sb = [big.tile([128, 512], BF16, tag=f"v{kt}", name=f"v{kt}") for kt in range(kt_tiles)]

            with tc.tile_pool(name="proj_psum", bufs=1, space="PSUM") as pp:
                # x tensors are fully resident; DMAs are issued in dependency
                # order (K first, then V, then Q in two halves) so each
                # projection stage's data lands just before it runs
                QC4 = DC // 4
                wk_sb = cpool.tile([128, DC, J], BF16, tag="wk")
                nc.sync.dma_start(wk_sb[:], wk.rearrange("(c p) j -> p c j", p=128))
                bk_sb = cpool.tile([128, JC], F32, tag="bk")
                nc.sync.dma_start(bk_sb[:], bk.rearrange("(c p) -> p c", p=128))
                xk_sb = big.tile([128, DC, k_pad], BF16, tag="xk")
                xk_r = xk.rearrange("(c p) k -> p c k", p=128)
                for q4 in range(4):
                    nc.sync.dma_start(
                        xk_sb[:, q4 * QC4 : (q4 + 1) * QC4, :],
                        xk_r[:, q4 * QC4 : (q4 + 1) * QC4, :],
                    )
                xv_sb = big.tile([128, DC, k_pad], BF16, tag="xv")
                nc.sync.dma_start(xv_sb[:], xv.rearrange("(c p) k -> p c k", p=128))
                wv_sb = cpool.tile([128, DC, J], BF16, tag="wv")
                nc.sync.dma_start(wv_sb[:], wv.rearrange("(c p) j -> p c j", p=128))
                bv_bc = cpool.tile([128, J], F32, tag="bv")
                nc.sync.dma_start(bv_bc[:], bv.ap()[None, :].to_broadcast((128, J)))
                kones_sb = cpool.tile([128, kt_tiles], F32, tag="kones")
                nc.sync.dma_start(kones_sb[:], kones.rearrange("(t p) -> p t", p=128))
                wq_sb = cpool.tile([128, DC, J], BF16, tag="wq")
                nc.sync.dma_start(wq_sb[:], wq.rearrange("(c p) j -> p c j", p=128))
                bq_sb = cpool.tile([128, JC], F32, tag="bq")
                nc.sync.dma_start(bq_sb[:], bq.rearrange("(c p) -> p c", p=128))
                xq_sb = big.tile([128, DC, S], BF16, tag="xq")
                xq_r = xq.rearrange("(c p) s -> p c s", p=128)
                for q4 in range(4):
                    nc.sync.dma_start(
                        xq_sb[:, q4 * QC4 : (q4 + 1) * QC4, :],
                        xq_r[:, q4 * QC4 : (q4 + 1) * QC4, :],
                    )
                wo_sb = cpool.tile([128, JC, D], BF16, tag="wo")
                nc.sync.dma_start(wo_sb[:], wo.rearrange("(c p) m -> p c m", p=128))

                # warm up the ACT exp table while ScalarE is otherwise
                # idle, so the ~2.7us first-use table load doesn't land in
                # the scalar-critical attention window
                warm = small.tile([1, 32], F32, tag="warm")
                nc.vector.memset(warm[:], 0.0)
                warm2 = small.tile([1, 32], F32, tag="warm")
                nc.scalar.activation(warm2[:], warm[:], AF.Exp, scale=1.0)

                # ---- K^T projection (merged per-chunk layout)
                kchunks = []
                off = 0
                while off < k_pad:
                    w = min(512, k_pad - off)
                    kchunks.append((off, w))
                    off += w
                psk = [
                    pp.tile([128, 512], F32, tag=f"psq{i}", name=f"psk{i}")
                    for i in range(JC * len(kchunks))
                ]
                for dc in range(DC):
                    for jc in range(JC):
                        lhsT = wk_sb[:, dc, jc * 128 : (jc + 1) * 128]
                        for i, (off, w) in enumerate(kchunks):
                            nc.tensor.matmul(
                                psk[jc * len(kchunks) + i][:, :w],
                                lhsT,
                                xk_sb[:, dc, off : off + w],
                                start=(dc == 0),
                                stop=(dc == DC - 1),
                            )
                for jc in range(JC):
                    for i, (off, w) in enumerate(kchunks):
                        nc.vector.tensor_tensor(
                            kt2[jc][:, off : off + w],
                            psk[jc * len(kchunks) + i][:, :w],
                            bk_sb[:, jc : jc + 1].to_broadcast((128, w)),
                            ALU.add,
                        )

                # ---- V natural projection (+ per-head 64-wide ones blocks) --
                for kt in range(kt_tiles):
                    psv = pp.tile([128, J], F32, tag=f"psq{kt % 2}", name="psv")
                    for dc in range(DC):
                        nc.tensor.matmul(
                            psv[:],
                            xv_sb[:, dc, kt * 128 : (kt + 1) * 128],
                            wv_sb[:, dc, :],
                            start=(dc == 0),
                            stop=(dc == DC - 1),
                        )
                    vt = v_sb[kt]
                    kcol = kones_sb[:, kt : kt + 1]
                    # pad keys have xv == 0, so psv pad rows are already 0:
                    # masking only needs to zero the bias on pad rows.
                    # bvko = bv * kones (rank-1), one op for all four heads.
                    bvko = small.tile([128, J], F32, tag="bvko")
                    nc.vector.tensor_scalar(
                        bvko[:], bv_bc[:], kcol, None, ALU.mult
                    )
                    for h in range(HL):
                        pair = h // 2
                        d0 = pair * 256 + (0 if h % 2 == 0 else 192)
                        nc.vector.tensor_tensor(
                            vt[:, d0 : d0 + DK],
                            psv[:, h * DK : (h + 1) * DK],
                            bvko[:, h * DK : (h + 1) * DK],
                            ALU.add,
                        )
                    for pair in range(JC):
                        o0 = pair * 256 + 64
                        nc.vector.tensor_copy(
                            vt[:, o0 : o0 + 128], kcol.to_broadcast((128, 128))
                        )


                # ---- Q^T projection (dc-outer; drains at the end)
                QQC = S // 512
                psq = [
                    pp.tile([128, 512], F32, tag=f"psq{i}", name=f"psq{i}")
                    for i in range(JC * QQC)
                ]
                for dc in range(DC):
                    for jc in range(JC):
                        lhsT = wq_sb[:, dc, jc * 128 : (jc + 1) * 128]
                        for qc in range(QQC):
                            nc.tensor.matmul(
                                psq[jc * QQC + qc][:],
                                lhsT,
                                xq_sb[:, dc, qc * 512 : (qc + 1) * 512],
                                start=(dc == 0),
                                stop=(dc == DC - 1),
                            )
                for qc in range(QQC):  # qc-major: granule 0 drains first
                    for jc in range(JC):
                        nc.vector.tensor_tensor(
                            qt_sb[jc][:, qc * 512 : (qc + 1) * 512],
                            psq[jc * QQC + qc][:],
                            bq_sb[:, jc : jc + 1].to_broadcast((128, 512)),
                            ALU.add,
                        )

            # ---- attention + per-granule output projection ------------------
            # Software-pipelined at (granule, pair)-block granularity: the
            # first two kt iterations of each block are emitted BEFORE the
            # previous block's psy/normalize tail, so ScalarE always has
            # buffered exp work while the in-order PE drains the lagged
            # out-projection tiles.
            with (
                tc.tile_pool(name="score_psum", bufs=2, space="PSUM") as sp,
                tc.tile_pool(name="aug_psum", bufs=4, space="PSUM") as ap,
            ):
                PIPE = 2 if kt_tiles > 3 else 0

                def att_kt(qg, pair, kt, aug_e, aug_o):
                    q0 = qg * QG
                    ps = sp.tile([128, 1024], F32, tag="ps", name="ps")
                    ksl = slice(kt * 128, (kt + 1) * 128)
                    # row-tiled concurrent score pair (K=64 each)
                    nc.tensor.matmul(
                        ps[:, 0:QG],
                        kt2[pair][0:64, ksl],
                        qt_sb[pair][0:64, q0 : q0 + QG],
                        start=True,
                        stop=True,
                    )
                    nc.tensor.matmul(
                        ps[:, QG : 2 * QG],
                        kt2[pair][64:128, ksl],
                        qt_sb[pair][64:128, q0 : q0 + QG],
                        start=True,
                        stop=True,
                    )
                    pt = ppool.tile([128, 1024], BF16, tag="pt")
                    nc.scalar.activation(pt[:], ps[:], AF.Exp, scale=0.125)
                    nc.tensor.matmul(
                        aug_e[:],
                        v_sb[kt][:, pair * 256 : pair * 256 + 128],
                        pt[:, 0:QG],
                        start=(kt == 0),
                        stop=(kt == kt_tiles - 1),
                    )
                    nc.tensor.matmul(
                        aug_o[:],
                        v_sb[kt][:, pair * 256 + 128 : pair * 256 + 256],
                        pt[:, QG : 2 * QG],
                        start=(kt == 0),
                        stop=(kt == kt_tiles - 1),
                    )

                def normalize(qg, pair, aug_e, aug_o):
                    # the custom reciprocal op only works at base partition 0
                    # on HW, but PLAIN DVE copies may cross halves — so the
                    # partition shifts are cheap vector copies, no DMAs
                    q0 = qg * QG
                    dl = small.tile([128, QG], F32, tag="dl")
                    nc.vector.tensor_copy(dl[0:64, :], aug_e[64:128, :])
                    rr = small.tile([128, QG], F32, tag="rr")
                    nc.vector.reciprocal_approx_fast(rr[0:64, :], dl[0:64, :])
                    ro = small.tile([128, QG], F32, tag="ro")
                    nc.vector.reciprocal_approx_fast(ro[0:64, :], aug_o[0:64, :])
                    rb = small.tile([128, QG], F32, tag="rb")
                    nc.vector.tensor_copy(rb[64:128, :], ro[0:64, :])
                    nc.vector.tensor_tensor(
                        at_sb[pair][0:64, q0 : q0 + QG],
                        aug_e[0:64, :],
                        rr[0:64, :],
                        ALU.mult,
                    )
                    nc.vector.tensor_tensor(
                        at_sb[pair][64:128, q0 : q0 + QG],
                        aug_o[64:128, :],
                        rb[64:128, :],
                        ALU.mult,
                    )

                prev = None  # (qg, pair, aug_e, aug_o) awaiting normalize
                for qg in range(NQG):
                    for pair in range(JC):
                        aug_e = ap.tile([128, QG], F32, tag="aug", name="aug_e")
                        aug_o = ap.tile([128, QG], F32, tag="aug", name="aug_o")
                        for kt in range(PIPE):
                            att_kt(qg, pair, kt, aug_e, aug_o)
                        if prev is not None:
                            pg, pp_, pe, po = prev
                            normalize(pg, pp_, pe, po)
                        for kt in range(PIPE, kt_tiles):
                            att_kt(qg, pair, kt, aug_e, aug_o)
                        prev = (qg, pair, aug_e, aug_o)
                pg, pp_, pe, po = prev
                normalize(pg, pp_, pe, po)
                # out-projection tail: attention is scalar-bound with no PE
                # slack, so any psy insert gaps the exp stream 1:1 — run the
                # whole projection as a PE-only tail instead
                y_r = y.rearrange("(t p) d -> p t d", p=128)
                for qg in range(NQG):
                    emit_outproj_granule(nc, ap, ypool, at_sb, wo_sb, y_r, qg)

    nc.finalize()
    return nc


_CACHE: dict = {}


def _get_program(kt_tiles: int):
    if kt_tiles not in _CACHE:
        _CACHE[kt_tiles] = build_program(kt_tiles)
    return _CACHE[kt_tiles]


def _prep_inputs(q, k, v, mask, Wq, bq, Wk, bk, Wv, bv, Wo, bo):
    """Shard + transpose + compact on the host. Returns (in_maps, kt_tiles)."""
    idx = [np.nonzero(mask[b])[0] for b in range(B)]
    s_u = max(1, max(len(i) for i in idx))
    kt_tiles = (s_u + 127) // 128
    k_pad = kt_tiles * 128

    per_batch = []
    for b in range(B):
        qT = np.ascontiguousarray(q[b].T).astype(BF16NP)  # [D, S]
        kT = np.zeros((D, k_pad), BF16NP)
        vT = np.zeros((D, k_pad), BF16NP)
        n = len(idx[b])
        kT[:, :n] = k[b].T[:, idx[b]].astype(BF16NP)
        vT[:, :n] = v[b].T[:, idx[b]].astype(BF16NP)
        ko = np.zeros((k_pad,), np.float32)
        ko[:n] = 1.0
        per_batch.append((qT, kT, vT, ko))

    in_maps = []
    for core in range(NCORES):
        b, g = divmod(core, GROUPS)
        j0 = g * J
        qT, kT, vT, ko = per_batch[b]
        in_maps.append(
            {
                "xq": qT,
                "xk": kT,
                "xv": vT,
                "wq": np.ascontiguousarray(Wq[j0 : j0 + J, :].T).astype(BF16NP),
                "wk": np.ascontiguousarray(Wk[j0 : j0 + J, :].T).astype(BF16NP),
                "wv": np.ascontiguousarray(Wv[j0 : j0 + J, :].T).astype(BF16NP),
                "wo": np.ascontiguousarray(Wo[:, j0 : j0 + J].T).astype(BF16NP),
                "bq": np.ascontiguousarray(bq[j0 : j0 + J]).astype(np.float32),
                "bk": np.ascontiguousarray(bk[j0 : j0 + J]).astype(np.float32),
                "bv": np.ascontiguousarray(bv[j0 : j0 + J]).astype(np.float32),
                "kones": ko,
            }
        )
    return in_maps, kt_tiles


def run(inputs: dict, trace: bool = False):
    """Run the sharded kernel; returns (output [B,S,D] f32, BassKernelResults)."""
    from concourse.bass_utils import run_bass_kernel_spmd

    inputs = {k: np.asarray(v) for k, v in inputs.items()}
    in_maps, kt_tiles = _prep_inputs(**inputs)
    nc = _get_program(kt_tiles)
    res = run_bass_kernel_spmd(nc, in_maps, list(range(NCORES)), trace=trace)
    bo = inputs["bo"].astype(np.float32)
    out = np.empty((B, S, D), np.float32)
    for b in range(B):
        acc = np.zeros((S, D), np.float64)
        for g in range(GROUPS):
            acc += np.asarray(res.results[b * GROUPS + g]["y"], dtype=np.float64)
        out[b] = (acc + bo[None, :]).astype(np.float32)
    return out, res


def kernel(**inputs) -> np.ndarray:
    out, _ = run(inputs, trace=False)
    return out
